# revision 14
# baseline (speedup 1.0000x reference)
"""TRN2 Bass kernel for nn_DFT: out = log((x @ Wr.T)^2 + (x @ Wi.T)^2).

x: [262144, 256] f32;  dft_real/dft_imag: [256, 256] f32 (symmetric DFT mats).

Strategy
--------
Data-parallel over 8 NeuronCores: each core handles 32768 rows (frames).

Math: x is real, so the spectrum is conjugate-symmetric: mag[b, k] ==
mag[b, 256-k]. The device computes only k = 0..128 (129 unique columns);
the host mirrors the rest. Additionally Im X_0 == Im X_128 == 0, so row 0
of the imaginary weight block is dead and is repurposed to carry the
k=128 real row — 129 outputs from a single pair of 128-row matmul chains.

Layout: device works in transposed (frequency-major) orientation.
Host passes xT = x.T per core ([256, 32768], contiguous); the PE computes
psum[p, n] = sum_j W[j, p] * xT[j, n] with the contraction (j) on the
partition axis, i.e. plain matmuls with no on-chip transposes. The host
transposes the [129, 32768] per-core result back and mirrors columns
129..255 from 127..1.

Per 512-column group: 2 input DMAs, 4 accumulating matmuls (2 K-chunks
x {real, imag}), squares on ScalarE (reading PSUM), sum on VectorE, Ln on
ScalarE, 2 output DMAs.
"""

import numpy as np

NFFT = 256
BATCH = 262144
N_CORES = 8
B_CORE = BATCH // N_CORES  # 32768
NB = 512                   # moving-dim tile (fp32 matmul max, one PSUM bank)
NG = B_CORE // NB          # 64 groups
NOUT = NFFT // 2 + 1       # 129 unique spectrum columns

# "fp32": exact, PE at 4 cycles/row (2 half-rate passes per matmul).
#   Measured: 243 us HW, absmax 3.6e-4 vs the fp32 reference. PE-bound,
#   100% PE busy — at the fp32-mode roofline.
# "split3": hi/lo float32r decomposition, 3 full-rate passes — near-fp32
#   accuracy (drops only the lo*lo term). Measured: 251 us, absmax 2.8e-2.
#   The on-device hi/lo extraction costs ~190 us of VectorE time, which
#   starves the PE (HAM re-throttles); it never beats fp32 in practice.
MODE = "split3"

_PROG_CACHE = {}


def _build_program(mode):
    import concourse.bacc as bacc
    import concourse.mybir as mybir
    import concourse.tile as tile

    mm_dt = mybir.dt.float32
    f32 = mybir.dt.float32

    nc = bacc.Bacc("TRN2", target_bir_lowering=False, debug=False)
    xT = nc.dram_tensor("xT", [NFFT, B_CORE], mm_dt, kind="ExternalInput").ap()
    w = nc.dram_tensor("w", [NFFT, NFFT], mm_dt, kind="ExternalInput").ap()
    outT = nc.dram_tensor("outT", [NOUT, B_CORE], f32, kind="ExternalOutput").ap()

    if mode == "split3":
        return _build_split3(nc, mybir, tile, xT, w, outT)

    Ln = mybir.ActivationFunctionType.Ln

    with tile.TileContext(nc) as tc:
        with (
            tc.tile_pool(name="wpool", bufs=1) as wpool,
            tc.tile_pool(name="xpool", bufs=4) as xpool,
            tc.tile_pool(name="pspool", bufs=4, space="PSUM") as pspool,
            tc.tile_pool(name="sqpool", bufs=4) as sqpool,
            tc.tile_pool(name="opool", bufs=4) as opool,
            tc.tile_pool(name="lpool", bufs=4) as lpool,
        ):
            # Weights resident for the whole kernel: w = [WrT | WiT'] with
            # rows j (contraction), cols 0:128 real / 128:256 imag.
            wt0 = wpool.tile([128, NFFT], mm_dt, tag="wt0")
            nc.sync.dma_start(wt0[:], w[0:128, :])
            wt1 = wpool.tile([128, NFFT], mm_dt, tag="wt1")
            nc.sync.dma_start(wt1[:], w[128:256, :])
            # Per-partition mask: 0 on partition 0 (whose imag slot carries
            # Re X_128, which must not leak into |X_0|^2), 1 elsewhere.
            mask = wpool.tile([128, 1], f32, tag="mask")
            nc.vector.memset(mask[:], 1.0)
            nc.vector.memset(mask[0:1, :], 0.0)

            for g in range(NG):
                cs = bass_ts(g, NB)
                x0 = xpool.tile([128, NB], mm_dt, tag="x0")
                nc.sync.dma_start(x0[:], xT[0:128, cs])
                x1 = xpool.tile([128, NB], mm_dt, tag="x1")
                nc.sync.dma_start(x1[:], xT[128:256, cs])

                ps_r = pspool.tile([128, NB], f32, tag="ps_r")
                nc.tensor.matmul(ps_r[:], wt0[:, 0:128], x0[:], start=True, stop=False)
                nc.tensor.matmul(ps_r[:], wt1[:, 0:128], x1[:], start=False, stop=True)
                ps_i = pspool.tile([128, NB], f32, tag="ps_i")
                nc.tensor.matmul(ps_i[:], wt0[:, 128:256], x0[:], start=True, stop=False)
                nc.tensor.matmul(ps_i[:], wt1[:, 128:256], x1[:], start=False, stop=True)

                sq_r = sqpool.tile([128, NB], f32, tag="sq_r")
                nc.scalar.square(sq_r[:], ps_r[:])
                sq_i = sqpool.tile([128, NB], f32, tag="sq_i")
                nc.scalar.square(sq_i[:], ps_i[:])

                o_last = lpool.tile([1, NB], f32, tag="o_last")
                nc.scalar.activation(o_last[:], sq_i[0:1, :], Ln)

                # |X_k|^2 = r^2 + mask*i^2 (mask kills the repurposed row 0).
                sq_f = sqpool.tile([128, NB], f32, tag="sq_f")
                nc.vector.scalar_tensor_tensor(
                    sq_f[:], sq_i[:], mask[:], sq_r[:],
                    op0=mybir.AluOpType.mult, op1=mybir.AluOpType.add,
                )

                o_main = opool.tile([128, NB], f32, tag="o_main")
                nc.scalar.activation(o_main[:], sq_f[:], Ln)

                nc.sync.dma_start(outT[0:128, cs], o_main[:])
                nc.sync.dma_start(outT[128:129, cs], o_last[:])

    nc.compile()
    return nc


def _build_split3(nc, mybir, tile, xT, w, outT):
    """x = xh + xl, W = wh + wl (float32r hi/lo); r = xh*wh + xl*wh + xh*wl.

    float32r matmuls run a single full-rate pass (vs 2 half-rate passes for
    fp32), so 3 passes beat fp32's effective 4. The hi/lo products are exact
    in the fp32 accumulator; only the lo*lo term (~2^-22 relative) is lost.
    Splitting happens on-device so the exact fp32r rounding width is
    irrelevant: xh = hw_round(x), xl = hw_round(x - xh).
    """
    f32 = mybir.dt.float32
    f32r = mybir.dt.float32r
    Ln = mybir.ActivationFunctionType.Ln
    A = mybir.AluOpType

    with tile.TileContext(nc) as tc:
        with (
            tc.tile_pool(name="wpool", bufs=1) as wpool,
            tc.tile_pool(name="xpool", bufs=6) as xpool,
            tc.tile_pool(name="xspool", bufs=8) as xspool,
            tc.tile_pool(name="pspool", bufs=4, space="PSUM") as pspool,
            tc.tile_pool(name="sqpool", bufs=4) as sqpool,
            tc.tile_pool(name="opool", bufs=4) as opool,
        ):
            wf, wh, wl = [], [], []
            for kc in range(2):
                wf_t = wpool.tile([128, NFFT], f32, tag=f"wf{kc}")
                nc.sync.dma_start(wf_t[:], w[kc * 128 : (kc + 1) * 128, :])
                wh_t = wpool.tile([128, NFFT], f32r, tag=f"wh{kc}")
                nc.vector.tensor_copy(wh_t[:], wf_t[:])
                wl_t = wpool.tile([128, NFFT], f32r, tag=f"wl{kc}")
                nc.vector.tensor_sub(wl_t[:], wf_t[:], wh_t[:])
                wf.append(wf_t); wh.append(wh_t); wl.append(wl_t)

            zrow = wpool.tile([1, NB], f32, tag="zrow")
            nc.vector.memset(zrow[:], 0.0)

            coll = wpool.tile([NG, NB], f32, tag="coll")

            for g in range(NG):
                cs = bass_ts(g, NB)
                xh, xl = [], []
                for kc in range(2):
                    x_t = xpool.tile([128, NB], f32, tag=f"x{kc}")
                    nc.sync.dma_start(x_t[:], xT[kc * 128 : (kc + 1) * 128, cs])
                    xh_t = xspool.tile([128, NB], f32r, tag=f"xh{kc}")
                    nc.vector.tensor_copy(xh_t[:], x_t[:])
                    xl_t = xspool.tile([128, NB], f32r, tag=f"xl{kc}")
                    nc.vector.tensor_sub(xl_t[:], x_t[:], xh_t[:])
                    xh.append(xh_t); xl.append(xl_t)

                ps = []
                for half in range(2):  # 0: real, 1: imag
                    wcol = bass_ts(half, 128)
                    p = pspool.tile([128, NB], f32, tag=f"ps{half}")
                    terms = []
                    for kc in range(2):
                        terms += [
                            (wh[kc], xh[kc]),
                            (wh[kc], xl[kc]),
                            (wl[kc], xh[kc]),
                        ]
                    for t, (wt, xt) in enumerate(terms):
                        nc.tensor.matmul(
                            p[:], wt[:, wcol], xt[:],
                            start=(t == 0), stop=(t == len(terms) - 1),
                        )
                    ps.append(p)

                sq_r = sqpool.tile([128, NB], f32, tag="sq_r")
                nc.scalar.square(sq_r[:], ps[0][:])
                sq_i = sqpool.tile([128, NB], f32, tag="sq_i")
                nc.scalar.square(sq_i[:], ps[1][:])

                # stash Re(X_128)^2 (row 0 of sq_i) for the batched tail Ln.
                # DMA, not an engine copy: engine writes must start at a
                # 32-aligned partition; DMA can target partition g directly.
                nc.sync.dma_start(coll[g : g + 1, :], sq_i[0:1, :])
                # zero the repurposed row, then plain add on the otherwise
                # idle GpSimd — keeps VectorE free for the hi/lo splits.
                nc.sync.dma_start(sq_i[0:1, :], zrow[:])
                sq_f = sqpool.tile([128, NB], f32, tag="sq_f")
                nc.gpsimd.tensor_add(sq_f[:], sq_i[:], sq_r[:])
                o_main = opool.tile([128, NB], f32, tag="o_main")
                nc.scalar.activation(o_main[:], sq_f[:], Ln)
                nc.sync.dma_start(outT[0:128, cs], o_main[:])

            o_coll = opool.tile([NG, NB], f32, tag="o_coll")
            nc.scalar.activation(o_coll[:], coll[:], Ln)
            out_last = outT[128:129, :].rearrange("a (g n) -> (a g) n", n=NB)
            nc.sync.dma_start(out_last, o_coll[:])

    nc.compile()
    return nc


def bass_ts(i, size):
    return slice(i * size, (i + 1) * size)


def _get_program(mode):
    if mode not in _PROG_CACHE:
        _PROG_CACHE[mode] = _build_program(mode)
    return _PROG_CACHE[mode]


def _make_weights(dft_real, dft_imag):
    wr_half = dft_real[0:128, :]
    wi_half = dft_imag[0:128, :].copy()
    wi_half[0, :] = dft_real[128, :]  # dead Im X_0 row carries Re X_128
    return np.concatenate([wr_half.T, wi_half.T], axis=1).astype(np.float32)


def _run(x, dft_real, dft_imag, trace=False, tmpdir=None):
    import concourse.bass_utils as bass_utils

    nc = _get_program(MODE)
    wfull = np.ascontiguousarray(_make_weights(dft_real, dft_imag))
    in_maps = []
    for c in range(N_CORES):
        xc = x[c * B_CORE : (c + 1) * B_CORE, :]
        in_maps.append({"xT": np.ascontiguousarray(xc.T), "w": wfull})
    res = bass_utils.run_bass_kernel_spmd(
        nc, in_maps, core_ids=list(range(N_CORES)), trace=trace, tmpdir=tmpdir
    )
    full = np.empty((BATCH, NFFT), dtype=np.float32)
    for c in range(N_CORES):
        block = res.results[c]["outT"]  # [129, B_CORE]
        full[c * B_CORE : (c + 1) * B_CORE, 0:NOUT] = block.T
    full[:, NOUT:NFFT] = full[:, NFFT - NOUT : 0 : -1]
    return full, res


def kernel(x, dft_real, dft_imag):
    x = np.asarray(x, dtype=np.float32)
    dft_real = np.asarray(dft_real, dtype=np.float32)
    dft_imag = np.asarray(dft_imag, dtype=np.float32)
    full, _ = _run(x, dft_real, dft_imag, trace=False)
    return full


# revision 15
# speedup vs baseline: 1.2974x; 1.2974x over previous
"""TRN2 Bass kernel for nn_DFT: out = log((x @ Wr.T)^2 + (x @ Wi.T)^2).

x: [262144, 256] f32;  dft_real/dft_imag: [256, 256] f32 (symmetric DFT mats).

Strategy
--------
Data-parallel over 8 NeuronCores: each core handles 32768 rows (frames).

Math: x is real, so the spectrum is conjugate-symmetric: mag[b, k] ==
mag[b, 256-k]. The device computes only k = 0..128 (129 unique columns);
the host mirrors the rest. Additionally Im X_0 == Im X_128 == 0, so row 0
of the imaginary weight block is dead and is repurposed to carry the
k=128 real row — 129 outputs from a single pair of 128-row matmul chains.

Layout: device works in transposed (frequency-major) orientation.
Host passes xT = x.T per core ([256, 32768], contiguous); the PE computes
psum[p, n] = sum_j W[j, p] * xT[j, n] with the contraction (j) on the
partition axis, i.e. plain matmuls with no on-chip transposes. The host
transposes the [129, 32768] per-core result back and mirrors columns
129..255 from 127..1.

Per 512-column group: 2 input DMAs, 4 accumulating matmuls (2 K-chunks
x {real, imag}), squares on ScalarE (reading PSUM), sum on VectorE, Ln on
ScalarE, 2 output DMAs.
"""

import numpy as np

NFFT = 256
BATCH = 262144
N_CORES = 8
B_CORE = BATCH // N_CORES  # 32768
NB = 512                   # moving-dim tile (fp32 matmul max, one PSUM bank)
NG = B_CORE // NB          # 64 groups
NOUT = NFFT // 2 + 1       # 129 unique spectrum columns

# "fp32": exact, PE at 4 cycles/row (2 half-rate passes per matmul).
#   Measured: 243 us HW, absmax 3.6e-4 vs the fp32 reference. PE-bound,
#   100% PE busy — at the fp32-mode roofline.
# "split3": hi/lo float32r decomposition, 3 full-rate passes — near-fp32
#   accuracy (drops only the lo*lo term). Measured: 251 us best, absmax
#   2.8e-2. The on-device hi/lo extraction costs ~190 us of VectorE time,
#   which starves the PE (HAM re-throttles). Offloading pieces to GpSimd
#   (casts: 380 us, mask-add: 312 us) only made it worse — Pool elementwise
#   is far slower than DVE. It never beats fp32 in practice.
MODE = "fp32"

_PROG_CACHE = {}


def _build_program(mode):
    import concourse.bacc as bacc
    import concourse.mybir as mybir
    import concourse.tile as tile

    mm_dt = mybir.dt.float32
    f32 = mybir.dt.float32

    nc = bacc.Bacc("TRN2", target_bir_lowering=False, debug=False)
    xT = nc.dram_tensor("xT", [NFFT, B_CORE], mm_dt, kind="ExternalInput").ap()
    w = nc.dram_tensor("w", [NFFT, NFFT], mm_dt, kind="ExternalInput").ap()
    outT = nc.dram_tensor("outT", [NOUT, B_CORE], f32, kind="ExternalOutput").ap()

    if mode == "split3":
        return _build_split3(nc, mybir, tile, xT, w, outT)

    Ln = mybir.ActivationFunctionType.Ln

    with tile.TileContext(nc) as tc:
        with (
            tc.tile_pool(name="wpool", bufs=1) as wpool,
            tc.tile_pool(name="xpool", bufs=4) as xpool,
            tc.tile_pool(name="pspool", bufs=4, space="PSUM") as pspool,
            tc.tile_pool(name="sqpool", bufs=4) as sqpool,
            tc.tile_pool(name="opool", bufs=4) as opool,
            tc.tile_pool(name="lpool", bufs=4) as lpool,
        ):
            # Weights resident for the whole kernel: w = [WrT | WiT'] with
            # rows j (contraction), cols 0:128 real / 128:256 imag.
            wt0 = wpool.tile([128, NFFT], mm_dt, tag="wt0")
            nc.sync.dma_start(wt0[:], w[0:128, :])
            wt1 = wpool.tile([128, NFFT], mm_dt, tag="wt1")
            nc.sync.dma_start(wt1[:], w[128:256, :])
            # Per-partition mask: 0 on partition 0 (whose imag slot carries
            # Re X_128, which must not leak into |X_0|^2), 1 elsewhere.
            mask = wpool.tile([128, 1], f32, tag="mask")
            nc.vector.memset(mask[:], 1.0)
            nc.vector.memset(mask[0:1, :], 0.0)

            for g in range(NG):
                cs = bass_ts(g, NB)
                x0 = xpool.tile([128, NB], mm_dt, tag="x0")
                nc.sync.dma_start(x0[:], xT[0:128, cs])
                x1 = xpool.tile([128, NB], mm_dt, tag="x1")
                nc.sync.dma_start(x1[:], xT[128:256, cs])

                ps_r = pspool.tile([128, NB], f32, tag="ps_r")
                nc.tensor.matmul(ps_r[:], wt0[:, 0:128], x0[:], start=True, stop=False)
                nc.tensor.matmul(ps_r[:], wt1[:, 0:128], x1[:], start=False, stop=True)
                ps_i = pspool.tile([128, NB], f32, tag="ps_i")
                nc.tensor.matmul(ps_i[:], wt0[:, 128:256], x0[:], start=True, stop=False)
                nc.tensor.matmul(ps_i[:], wt1[:, 128:256], x1[:], start=False, stop=True)

                sq_r = sqpool.tile([128, NB], f32, tag="sq_r")
                nc.scalar.square(sq_r[:], ps_r[:])
                sq_i = sqpool.tile([128, NB], f32, tag="sq_i")
                nc.scalar.square(sq_i[:], ps_i[:])

                o_last = lpool.tile([1, NB], f32, tag="o_last")
                nc.scalar.activation(o_last[:], sq_i[0:1, :], Ln)

                # |X_k|^2 = r^2 + mask*i^2 (mask kills the repurposed row 0).
                sq_f = sqpool.tile([128, NB], f32, tag="sq_f")
                nc.vector.scalar_tensor_tensor(
                    sq_f[:], sq_i[:], mask[:], sq_r[:],
                    op0=mybir.AluOpType.mult, op1=mybir.AluOpType.add,
                )

                o_main = opool.tile([128, NB], f32, tag="o_main")
                nc.scalar.activation(o_main[:], sq_f[:], Ln)

                nc.sync.dma_start(outT[0:128, cs], o_main[:])
                nc.sync.dma_start(outT[128:129, cs], o_last[:])

    nc.compile()
    return nc


def _build_split3(nc, mybir, tile, xT, w, outT):
    """x = xh + xl, W = wh + wl (float32r hi/lo); r = xh*wh + xl*wh + xh*wl.

    float32r matmuls run a single full-rate pass (vs 2 half-rate passes for
    fp32), so 3 passes beat fp32's effective 4. The hi/lo products are exact
    in the fp32 accumulator; only the lo*lo term (~2^-22 relative) is lost.
    Splitting happens on-device so the exact fp32r rounding width is
    irrelevant: xh = hw_round(x), xl = hw_round(x - xh).
    """
    f32 = mybir.dt.float32
    f32r = mybir.dt.float32r
    Ln = mybir.ActivationFunctionType.Ln
    A = mybir.AluOpType

    with tile.TileContext(nc) as tc:
        with (
            tc.tile_pool(name="wpool", bufs=1) as wpool,
            tc.tile_pool(name="xpool", bufs=6) as xpool,
            tc.tile_pool(name="xspool", bufs=8) as xspool,
            tc.tile_pool(name="pspool", bufs=4, space="PSUM") as pspool,
            tc.tile_pool(name="sqpool", bufs=4) as sqpool,
            tc.tile_pool(name="opool", bufs=4) as opool,
        ):
            wf, wh, wl = [], [], []
            for kc in range(2):
                wf_t = wpool.tile([128, NFFT], f32, tag=f"wf{kc}")
                nc.sync.dma_start(wf_t[:], w[kc * 128 : (kc + 1) * 128, :])
                wh_t = wpool.tile([128, NFFT], f32r, tag=f"wh{kc}")
                nc.vector.tensor_copy(wh_t[:], wf_t[:])
                wl_t = wpool.tile([128, NFFT], f32r, tag=f"wl{kc}")
                nc.vector.tensor_sub(wl_t[:], wf_t[:], wh_t[:])
                wf.append(wf_t); wh.append(wh_t); wl.append(wl_t)

            mask = wpool.tile([128, 1], f32, tag="mask")
            nc.vector.memset(mask[:], 1.0)
            nc.vector.memset(mask[0:1, :], 0.0)

            coll = wpool.tile([NG, NB], f32, tag="coll")

            for g in range(NG):
                cs = bass_ts(g, NB)
                xh, xl = [], []
                for kc in range(2):
                    x_t = xpool.tile([128, NB], f32, tag=f"x{kc}")
                    nc.sync.dma_start(x_t[:], xT[kc * 128 : (kc + 1) * 128, cs])
                    xh_t = xspool.tile([128, NB], f32r, tag=f"xh{kc}")
                    nc.vector.tensor_copy(xh_t[:], x_t[:])
                    xl_t = xspool.tile([128, NB], f32r, tag=f"xl{kc}")
                    nc.vector.tensor_sub(xl_t[:], x_t[:], xh_t[:])
                    xh.append(xh_t); xl.append(xl_t)

                ps = []
                for half in range(2):  # 0: real, 1: imag
                    wcol = bass_ts(half, 128)
                    p = pspool.tile([128, NB], f32, tag=f"ps{half}")
                    terms = []
                    for kc in range(2):
                        terms += [
                            (wh[kc], xh[kc]),
                            (wh[kc], xl[kc]),
                            (wl[kc], xh[kc]),
                        ]
                    for t, (wt, xt) in enumerate(terms):
                        nc.tensor.matmul(
                            p[:], wt[:, wcol], xt[:],
                            start=(t == 0), stop=(t == len(terms) - 1),
                        )
                    ps.append(p)

                sq_r = sqpool.tile([128, NB], f32, tag="sq_r")
                nc.scalar.square(sq_r[:], ps[0][:])
                sq_i = sqpool.tile([128, NB], f32, tag="sq_i")
                nc.scalar.square(sq_i[:], ps[1][:])

                # stash Re(X_128)^2 (row 0 of sq_i) for the batched tail Ln.
                # DMA, not an engine copy: engine writes must start at a
                # 32-aligned partition; DMA can target partition g directly.
                nc.sync.dma_start(coll[g : g + 1, :], sq_i[0:1, :])
                sq_f = sqpool.tile([128, NB], f32, tag="sq_f")
                nc.vector.scalar_tensor_tensor(
                    sq_f[:], sq_i[:], mask[:], sq_r[:], op0=A.mult, op1=A.add
                )
                o_main = opool.tile([128, NB], f32, tag="o_main")
                nc.scalar.activation(o_main[:], sq_f[:], Ln)
                nc.sync.dma_start(outT[0:128, cs], o_main[:])

            o_coll = opool.tile([NG, NB], f32, tag="o_coll")
            nc.scalar.activation(o_coll[:], coll[:], Ln)
            out_last = outT[128:129, :].rearrange("a (g n) -> (a g) n", n=NB)
            nc.sync.dma_start(out_last, o_coll[:])

    nc.compile()
    return nc


def bass_ts(i, size):
    return slice(i * size, (i + 1) * size)


def _get_program(mode):
    if mode not in _PROG_CACHE:
        _PROG_CACHE[mode] = _build_program(mode)
    return _PROG_CACHE[mode]


def _make_weights(dft_real, dft_imag):
    wr_half = dft_real[0:128, :]
    wi_half = dft_imag[0:128, :].copy()
    wi_half[0, :] = dft_real[128, :]  # dead Im X_0 row carries Re X_128
    return np.concatenate([wr_half.T, wi_half.T], axis=1).astype(np.float32)


def _run(x, dft_real, dft_imag, trace=False, tmpdir=None):
    import concourse.bass_utils as bass_utils

    nc = _get_program(MODE)
    wfull = np.ascontiguousarray(_make_weights(dft_real, dft_imag))
    in_maps = []
    for c in range(N_CORES):
        xc = x[c * B_CORE : (c + 1) * B_CORE, :]
        in_maps.append({"xT": np.ascontiguousarray(xc.T), "w": wfull})
    res = bass_utils.run_bass_kernel_spmd(
        nc, in_maps, core_ids=list(range(N_CORES)), trace=trace, tmpdir=tmpdir
    )
    full = np.empty((BATCH, NFFT), dtype=np.float32)
    for c in range(N_CORES):
        block = res.results[c]["outT"]  # [129, B_CORE]
        full[c * B_CORE : (c + 1) * B_CORE, 0:NOUT] = block.T
    full[:, NOUT:NFFT] = full[:, NFFT - NOUT : 0 : -1]
    return full, res


def kernel(x, dft_real, dft_imag):
    x = np.asarray(x, dtype=np.float32)
    dft_real = np.asarray(dft_real, dtype=np.float32)
    dft_imag = np.asarray(dft_imag, dtype=np.float32)
    full, _ = _run(x, dft_real, dft_imag, trace=False)
    return full
